# revision 34
# baseline (speedup 1.0000x reference)
"""MultiHeadLinearAttention Trainium2 Bass kernel — 8-core SPMD (v2).

Problem (per reference):
  q = elu(LN(Xq @ Wq.T + bq)) + 1 ; k = elu(LN(Xk @ Wk.T + bk)) + 1
  v = Xv @ Wv.T + bv
  kv = sum_n k[n] (x) v[n]   (per head, [D,D]);  ksum = sum_n k[n]
  out = ((q @ kv) / (q . ksum + 1e-8)) @ Wo.T + bo

Sharding: core c -> batch b = c//2, token half h = c%2 (2048 q AND k/v
tokens each). Per-pair (cores 2b, 2b+1) AllReduce of kv/ksum partials
(~266 KB) completes the sum over all 4096 k/v tokens of the batch.

Layouts on chip (per core):
  k,v: [tok x feat] (LN over free dim; kv contraction over token partitions)
  q:   [feat x tok] (q^T feeds num = kv_bd^T @ q^T and out-proj lhsT)
LayerNorm mean is folded into the weights on host (W~ = W^T(I-J/E),
b~ = b - mean(b)); gq/gk==1, betaq/betak==0 (asserted) so
LN(y) = (y - mu(y)) * rsqrt(var + eps) = u * exp(-0.5*ln(mean(u^2)+eps)).
elu(z)+1 = exp(min(z,0)) + relu(z).

v2 structure (vs v1): Phase A uses [128,512] PSUM proj chunks (bufs=2)
and per-tile rstd so elu/kv pipeline per 128-token tile; the q
projection (B1, AR-independent) is emitted entirely before any
AR-dependent op so the AllReduce hides under it; B2 (elu+den+num) and
the out-projection C are pipelined per 512-token chunk with the den
division fused into the num PSUM evacuation.

All matmuls run as float32r (FP32 bits read at FP22 precision, full PE
rate at moving-dim >= 256).
"""

import os

import numpy as np

B, NSEQ, E, H, D = 4, 4096, 1024, 16, 64
NCORES = 8
T = NSEQ // 2          # tokens per core
TT = T // 128          # token tiles (16)
EI = E // 128          # feature tiles (8)
S = 4                  # B-phase token chunks
SC = T // S            # 512 tokens per chunk
LN_EPS = 1e-5

_NC_CACHE = {}


def _build_nc():
    from concourse import bacc
    import concourse.bass as bass
    import concourse.mybir as mybir
    import concourse.tile as tile

    f32 = mybir.dt.float32
    f32r = mybir.dt.float32r
    Alu = mybir.AluOpType
    Act = mybir.ActivationFunctionType
    RG = [[0, 1], [2, 3], [4, 5], [6, 7]]

    nc = bacc.Bacc(num_devices=NCORES)

    xqT = nc.dram_tensor("xqT", [E, T], f32r, kind="ExternalInput")
    xkT = nc.dram_tensor("xkT", [E, T], f32r, kind="ExternalInput")
    xvT = nc.dram_tensor("xvT", [E, T], f32r, kind="ExternalInput")
    wqT = nc.dram_tensor("wqT", [E, E], f32r, kind="ExternalInput")
    wkT = nc.dram_tensor("wkT", [E, E], f32r, kind="ExternalInput")
    wvT = nc.dram_tensor("wvT", [E, E], f32r, kind="ExternalInput")
    woT = nc.dram_tensor("woT", [E, E], f32r, kind="ExternalInput")
    bq2d = nc.dram_tensor("bq2d", [128, EI], f32, kind="ExternalInput")
    bkR = nc.dram_tensor("bkR", [1, E], f32r, kind="ExternalInput")
    bvR = nc.dram_tensor("bvR", [1, E], f32r, kind="ExternalInput")
    onesC = nc.dram_tensor("onesC", [128, 1], f32r, kind="ExternalInput")
    e2R = nc.dram_tensor("e2R", [2, 128], f32r, kind="ExternalInput")
    zerosBD = nc.dram_tensor("zerosBD", [128, E], f32r, kind="ExternalInput")
    out_d = nc.dram_tensor("out", [T, E], f32, kind="ExternalOutput")

    xqT_v = xqT.rearrange("(i p) n -> p i n", p=128)
    xkT_v = xkT.rearrange("(i p) n -> p i n", p=128)
    xvT_v = xvT.rearrange("(i p) n -> p i n", p=128)
    wqT_v = wqT.rearrange("(i p) j -> p i j", p=128)
    wkT_v = wkT.rearrange("(i p) j -> p i j", p=128)
    wvT_v = wvT.rearrange("(i p) j -> p i j", p=128)
    woT_v = woT.rearrange("(e p) j -> p e j", p=128)

    with tile.TileContext(nc) as tc:
        with tc.tile_pool(name="const", bufs=1) as cp, \
             tc.tile_pool(name="dram", bufs=1, space="DRAM") as dp:
            onesC_sb = cp.tile([128, 1], f32r, tag="onesC_sb")
            nc.sync.dma_start(out=onesC_sb, in_=onesC[:, :])
            e2_sb = cp.tile([2, 128], f32r, tag="e2_sb")
            nc.sync.dma_start(out=e2_sb, in_=e2R[:, :])
            zrow_sb = cp.tile([1, 520], f32r, tag="zrow_sb")
            nc.sync.dma_start(out=zrow_sb, in_=zerosBD[0:1, 0:520])
            ones_row = cp.tile([1, 128], f32, tag="ones_row")
            nc.vector.memset(ones_row, 1.0)
            onesR = ones_row.bitcast(f32r)
            eps_sb = cp.tile([128, 1], f32, tag="eps_sb")
            nc.vector.memset(eps_sb, LN_EPS)
            eps_row = cp.tile([1, 1], f32, tag="eps_row")
            nc.vector.memset(eps_row, LN_EPS)
            bq_sb = cp.tile([128, EI], f32, tag="bq_sb")
            nc.sync.dma_start(out=bq_sb, in_=bq2d[:, :])
            bk_b = cp.tile([128, E], f32r, tag="bk_b")
            nc.sync.dma_start(out=bk_b, in_=bkR[:, :].to_broadcast([128, E]))
            bv_b = cp.tile([128, E], f32r, tag="bv_b")
            nc.sync.dma_start(out=bv_b, in_=bvR[:, :].to_broadcast([128, E]))
            kvbd = cp.tile([128, E], f32r, tag="kvbd")
            ksum2 = cp.tile([128, 16], f32r, tag="ksum2")
            ar_sb = cp.tile([128, 520], f32, tag="ar_sb")
            cc_in = dp.tile([128, 520], f32, tag="cc_in")
            cc_out = dp.tile([128, 520], f32, tag="cc_out")


            # ============ Phase A: k/v proj + elu + kv, per-tile pipeline =
            with tc.tile_pool(name="paw", bufs=1) as paw, \
                 tc.tile_pool(name="pax", bufs=2) as pax, \
                 tc.tile_pool(name="par", bufs=2) as par, \
                 tc.tile_pool(name="psA", bufs=1, space="PSUM") as psA, \
                 tc.tile_pool(name="psAp", bufs=2, space="PSUM") as psAp:
                wk_sb = paw.tile([128, EI, E], f32r, tag="wk")
                for i in range(EI):
                    nc.sync.dma_start(out=wk_sb[:, i, :], in_=wkT_v[:, i, :])
                wv_sb = paw.tile([128, EI, E], f32r, tag="wv")
                ss_all = paw.tile([128, TT], f32, tag="ss_all")
                rstd_all = paw.tile([128, TT], f32, tag="rstd_all")

                kv_ps = [psA.tile([128, 512], f32, tag=f"kv{q}",
                                  name=f"kv{q}") for q in range(4)]
                ksum_ps = psA.tile([128, 8], f32, tag="ksum")
                for q in range(4):
                    nc.tensor.matmul(kv_ps[q], onesR, zrow_sb[:, 0:512],
                                     start=True, stop=False,
                                     skip_group_check=True)
                nc.tensor.matmul(ksum_ps, onesR, zrow_sb[:, 0:8],
                                 start=True, stop=False, skip_group_check=True)

                XC = 256  # tokens per x chunk (2 tiles)
                for g in range(4):          # rstd batch group: 4 tiles
                    ku_t = {}
                    vu_t = {}
                    for cc in range(2):
                        qq = 2 * g + cc
                        qsl = slice(XC * qq, XC * qq + XC)
                        xk_q = pax.tile([128, EI, XC], f32r, tag="xk")
                        for i in range(EI):
                            nc.sync.dma_start(out=xk_q[:, i, :],
                                              in_=xkT_v[:, i, qsl])
                        xv_q = pax.tile([128, EI, XC], f32r, tag="xv")
                        for i in range(EI):
                            nc.sync.dma_start(out=xv_q[:, i, :],
                                              in_=xvT_v[:, i, qsl])
                        if qq == 0:
                            # wv after first xk/xv so tile-0 k MMs start early
                            for i in range(EI):
                                nc.sync.dma_start(out=wv_sb[:, i, :],
                                                  in_=wvT_v[:, i, :])
                        for tl in range(2):
                            t = 2 * qq + tl
                            tsl = slice(128 * tl, 128 * tl + 128)
                            ku = par.tile([128, E], f32, tag="ku", bufs=5,
                                          name=f"ku{t}")
                            kps = psAp.tile([128, E], f32, tag="kps",
                                            bufs=1)
                            for i in range(EI):
                                for jh in range(2):
                                    js = slice(512 * jh, 512 * jh + 512)
                                    nc.tensor.matmul(kps[:, js],
                                                     xk_q[:, i, tsl],
                                                     wk_sb[:, i, js],
                                                     start=(i == 0),
                                                     stop=(i == EI - 1))
                            nc.vector.scalar_tensor_tensor(
                                out=ku, in0=kps, scalar=1.0,
                                in1=bk_b.bitcast(f32),
                                op0=Alu.mult, op1=Alu.add)
                            vu = par.tile([128, E], f32r, tag="vu", bufs=5,
                                          name=f"vu{t}")
                            for jh in range(2):
                                js = slice(512 * jh, 512 * jh + 512)
                                vps = psAp.tile([128, 512], f32, tag="vps",
                                                bufs=1)
                                for i in range(EI):
                                    nc.tensor.matmul(vps, xv_q[:, i, tsl],
                                                     wv_sb[:, i, js],
                                                     start=(i == 0),
                                                     stop=(i == EI - 1))
                                nc.vector.scalar_tensor_tensor(
                                    out=vu[:, js], in0=vps,
                                    scalar=1.0, in1=bv_b.bitcast(f32)[:, js],
                                    op0=Alu.mult, op1=Alu.add)
                            scrap = par.tile([128, E], f32, tag="scrap",
                                             bufs=1)
                            nc.scalar.activation(out=scrap, in_=ku,
                                                 func=Act.Square,
                                                 accum_out=ss_all[:, t:t + 1])
                            ku_t[t] = ku
                            vu_t[t] = vu
                    gs = slice(4 * g, 4 * g + 4)
                    nc.scalar.activation(out=rstd_all[:, gs],
                                         in_=ss_all[:, gs], func=Act.Ln,
                                         scale=1.0 / E, bias=eps_sb)
                    nc.scalar.activation(out=rstd_all[:, gs],
                                         in_=rstd_all[:, gs], func=Act.Exp,
                                         scale=-0.5)
                    for t in range(4 * g, 4 * g + 4):
                        ku = ku_t[t]
                        vu = vu_t[t]
                        rs = rstd_all[:, t:t + 1]
                        km = par.tile([128, E], f32, tag="km")
                        nc.vector.tensor_scalar(out=km, in0=ku, scalar1=rs,
                                                scalar2=0.0, op0=Alu.mult,
                                                op1=Alu.min)
                        nc.scalar.activation(out=km, in_=km, func=Act.Exp)
                        kf = par.tile([128, E], f32r, tag="kf", bufs=3)
                        nc.scalar.activation(out=kf, in_=ku, func=Act.Relu,
                                             scale=rs)
                        nc.vector.tensor_tensor(out=kf, in0=kf.bitcast(f32),
                                                in1=km, op=Alu.add)
                        for q4 in range(4):
                            vq = vu[:, 256 * q4:256 * q4 + 256]
                            for hf in range(2):
                                pr = 2 * q4 + hf
                                kp = kf[:, 128 * pr:128 * pr + 128]
                                nc.tensor.matmul(
                                    kv_ps[q4][:, 256 * hf:256 * hf + 256],
                                    kp, vq, start=False, stop=(t == TT - 1),
                                    skip_group_check=True)
                                nc.tensor.matmul(
                                    ksum_ps[:, pr:pr + 1], kp.bitcast(f32),
                                    onesC_sb.bitcast(f32),
                                    start=False, stop=(t == TT - 1),
                                    skip_group_check=True)

                pack = paw.tile([128, 520], f32, tag="pack")
                for p in range(8):
                    q4, odd = divmod(p, 2)
                    c = 64 * p
                    if odd == 0:
                        nc.vector.tensor_copy(out=pack[0:64, c:c + 64],
                                              in_=kv_ps[q4][0:64, 0:64])
                        nc.vector.tensor_copy(out=pack[64:128, c:c + 64],
                                              in_=kv_ps[q4][64:128, 64:128])
                    else:
                        nc.vector.tensor_copy(out=pack[0:64, c:c + 64],
                                              in_=kv_ps[q4][0:64, 384:448])
                        nc.vector.tensor_copy(out=pack[64:128, c:c + 64],
                                              in_=kv_ps[q4][64:128, 448:512])
                nc.vector.tensor_copy(out=pack[:, 512:520], in_=ksum_ps[:, :])
                nc.sync.dma_start(out=cc_in, in_=pack)

            nc.gpsimd.collective_compute(
                "AllReduce", Alu.add, replica_groups=RG,
                ins=[cc_in[:, :]], outs=[cc_out[:, :]])

            # ============ Phase B1: q proj + LN stats (AR-independent) ====
            # u tiles per (j, s) so B2 writes and C reads stay chunk-local.
            with tc.tile_pool(name="pu", bufs=1) as pu:
                u_t = [[pu.tile([128, SC], f32r, tag=f"u{j}_{s}",
                                name=f"u{j}_{s}") for s in range(S)]
                       for j in range(EI)]
                rb_t = [pu.tile([128, SC], f32, tag=f"rb{s}", name=f"rb{s}")
                        for s in range(S)]
                wo_sb = pu.tile([128, EI, E], f32r, tag="wo")

                with tc.tile_pool(name="pbw", bufs=1) as pbw, \
                     tc.tile_pool(name="pbx", bufs=2) as pbx, \
                     tc.tile_pool(name="pb1", bufs=2) as pb1, \
                     tc.tile_pool(name="psB1", bufs=2, space="PSUM") as psB1:
                    wq_sb = pbw.tile([128, EI, E], f32r, tag="wq")
                    for i in range(EI):
                        nc.sync.dma_start(out=wq_sb[:, i, :],
                                          in_=wqT_v[:, i, :])
                    ssq_list = []
                    for s in range(S):
                        ssl = slice(SC * s, SC * s + SC)
                        xq_s = pbx.tile([128, EI, SC], f32r, tag="xq")
                        for i in range(EI):
                            nc.sync.dma_start(out=xq_s[:, i, :],
                                              in_=xqT_v[:, i, ssl])
                        if s == 1:
                            # wo loads during B1 so C(0) doesn't wait on it
                            for e in range(EI):
                                nc.sync.dma_start(out=wo_sb[:, e, :],
                                                  in_=woT_v[:, e, :])
                        ssq_ps = psB1.tile([1, 512], f32, tag=f"ssq{s}",
                                           bufs=1, name=f"ssq{s}")
                        for j in range(EI):
                            qps = psB1.tile([128, 512], f32, tag="qps")
                            for i in range(EI):
                                nc.tensor.matmul(
                                    qps, wq_sb[:, i, 128 * j:128 * j + 128],
                                    xq_s[:, i, :], start=(i == 0),
                                    stop=(i == EI - 1))
                            u = u_t[j][s]
                            nc.vector.tensor_scalar_add(
                                out=u, in0=qps,
                                scalar1=bq_sb[:, j:j + 1])
                            usq = pb1.tile([128, 512], f32r, tag="usq")
                            nc.scalar.activation(out=usq,
                                                 in_=qps, func=Act.Square,
                                                 bias=bq_sb[:, j:j + 1])
                            nc.tensor.matmul(ssq_ps, onesC_sb, usq,
                                             start=(j == 0),
                                             stop=(j == EI - 1),
                                             skip_group_check=True)
                        ssq_list.append(ssq_ps)
                    # batched rstd for all chunks: one Ln/Exp table trip,
                    # then PE K=1 broadcast to 128 partitions
                    ln_row = pb1.tile([1, T], f32, tag="ln_row", bufs=1)
                    rr_all = pb1.tile([1, T], f32r, tag="rr_all", bufs=1)
                    for s in range(S):
                        nc.scalar.activation(out=ln_row[:, SC * s:SC * s + SC],
                                             in_=ssq_list[s], func=Act.Ln,
                                             scale=1.0 / E, bias=eps_row)
                    nc.scalar.activation(out=rr_all, in_=ln_row,
                                         func=Act.Exp, scale=-0.5)
                    for s in range(S):
                        ssl = slice(SC * s, SC * s + SC)
                        rb_ps = psB1.tile([128, 512], f32, tag="rbps",
                                          bufs=2)
                        nc.tensor.matmul(rb_ps, onesR, rr_all[:, ssl],
                                         start=True, stop=True)
                        nc.vector.tensor_copy(out=rb_t[s], in_=rb_ps)

                # ====== AR unpack (first AR-dependent ops) ======
                nc.gpsimd.dma_start(out=ar_sb, in_=cc_out[:, :])
                nc.gpsimd.dma_start(out=kvbd, in_=zerosBD[:, :])
                nc.gpsimd.dma_start(out=ksum2, in_=zerosBD[:, 0:16])
                ev_dst = kvbd[0:64, :].rearrange("p (a two c) -> p a two c",
                                                 two=2, c=64)[:, :, 0, :]
                nc.vector.tensor_copy(
                    out=ev_dst,
                    in_=ar_sb[0:64, 0:512].rearrange(
                        "p (a c) -> p a c", c=64).bitcast(f32r))
                od_dst = kvbd[64:128, :].rearrange("p (a two c) -> p a two c",
                                                   two=2, c=64)[:, :, 1, :]
                nc.vector.tensor_copy(
                    out=od_dst,
                    in_=ar_sb[64:128, 0:512].rearrange(
                        "p (a c) -> p a c", c=64).bitcast(f32r))
                for jj in range(EI):
                    nc.vector.tensor_copy(
                        out=ksum2[0:64, 2 * jj:2 * jj + 1],
                        in_=ar_sb[0:64, 512 + jj:513 + jj].bitcast(f32r))
                    nc.vector.tensor_copy(
                        out=ksum2[64:128, 2 * jj + 1:2 * jj + 2],
                        in_=ar_sb[64:128, 512 + jj:513 + jj].bitcast(f32r))

                # ============ Phase B2 + C: per-chunk pipeline ============
                with tc.tile_pool(name="pc2", bufs=3) as pc2, \
                     tc.tile_pool(name="psB2", bufs=2, space="PSUM") as psB2:
                    for s in range(S):
                        for j in range(EI):
                            u = u_t[j][s]
                            qf = u.bitcast(f32)
                            nc.vector.tensor_tensor(out=u, in0=qf,
                                                    in1=rb_t[s], op=Alu.mult)
                            m = pc2.tile([128, SC], f32, tag="m")
                            nc.gpsimd.tensor_scalar_min(out=m, in0=qf,
                                                        scalar1=0.0)
                            nc.scalar.activation(out=m, in_=m, func=Act.Exp)
                            nc.scalar.activation(out=u, in_=qf,
                                                 func=Act.Relu)
                            nc.vector.tensor_tensor(out=u, in0=qf, in1=m,
                                                    op=Alu.add)
                            den_ps = psB2.tile([2, 512], f32, tag="dps")
                            nc.tensor.matmul(den_ps, ksum2[:, 2 * j:2 * j + 2],
                                             u, start=True, stop=True)
                            den_r = pc2.tile([2, 512], f32r, tag="denr")
                            nc.vector.tensor_copy(out=den_r, in_=den_ps)
                            drb_ps = psB2.tile([128, 512], f32, tag="drbps")
                            nc.tensor.matmul(drb_ps, e2_sb, den_r,
                                             start=True, stop=True)
                            drb = pc2.tile([128, SC], f32, tag="drb")
                            nc.vector.reciprocal_approx_fast(out=drb,
                                                             in_=drb_ps)
                            num_ps = psB2.tile([128, 512], f32, tag="nps")
                            nc.tensor.matmul(num_ps, kvbd[:, 128 * j:128 * j + 128],
                                             u, start=True, stop=True)
                            nc.vector.tensor_tensor(out=u,
                                                    in0=num_ps, in1=drb,
                                                    op=Alu.mult)
                        for tl in range(4):
                            tsl = slice(128 * tl, 128 * tl + 128)
                            gtsl = slice(SC * s + 128 * tl,
                                         SC * s + 128 * tl + 128)
                            for jh in range(2):
                                js = slice(512 * jh, 512 * jh + 512)
                                ops = psB2.tile([128, 512], f32, tag="ops")
                                for e in range(EI):
                                    nc.tensor.matmul(
                                        ops, u_t[e][s][:, tsl],
                                        wo_sb[:, e, js], start=(e == 0),
                                        stop=(e == EI - 1))
                                osb = pc2.tile([128, 512], f32, tag="osb")
                                nc.scalar.activation(out=osb, in_=ops,
                                                     func=Act.Copy)
                                nc.sync.dma_start(out=out_d[gtsl, js],
                                                  in_=osb)

    nc.finalize()
    return nc


def _prep_inputs(inputs):
    """Host-side fold + per-core shard maps."""
    f = np.float32
    Wq, bq = inputs["Wq"], inputs["bq"]
    Wk, bk = inputs["Wk"], inputs["bk"]
    Wv, bv = inputs["Wv"], inputs["bv"]
    Wo, bo = inputs["Wo"], inputs["bo"]
    for name in ("gq", "gk"):
        assert np.allclose(np.asarray(inputs[name]), 1.0), f"{name} != 1 unsupported"
    for name in ("betaq", "betak"):
        assert np.allclose(np.asarray(inputs[name]), 0.0), f"{name} != 0 unsupported"

    wqT = np.ascontiguousarray(np.asarray(Wq, f).T)
    wqT = wqT - wqT.mean(axis=1, keepdims=True)
    bqf = np.asarray(bq, f) - np.asarray(bq, f).mean()
    wkT = np.ascontiguousarray(np.asarray(Wk, f).T)
    wkT = wkT - wkT.mean(axis=1, keepdims=True)
    bkf = np.asarray(bk, f) - np.asarray(bk, f).mean()
    wvT = np.ascontiguousarray(np.asarray(Wv, f).T)
    woT = np.ascontiguousarray(np.asarray(Wo, f).T)

    shared = {
        "wqT": np.ascontiguousarray(wqT, f),
        "wkT": np.ascontiguousarray(wkT, f),
        "wvT": wvT,
        "woT": woT,
        "bq2d": np.ascontiguousarray(bqf.reshape(EI, 128).T, f),
        "bkR": np.ascontiguousarray(bkf.reshape(1, E), f),
        "bvR": np.ascontiguousarray(np.asarray(bv, f).reshape(1, E)),
        "onesC": np.ones((128, 1), f),
        "e2R": np.ascontiguousarray(
            np.repeat(np.eye(2, dtype=f), 64, axis=1)),
        "zerosBD": np.zeros((128, E), f),
    }
    qe = np.asarray(inputs["query_embed"], f)
    ke = np.asarray(inputs["key_embed"], f)
    ve = np.asarray(inputs["value"], f)
    in_maps = []
    for c in range(NCORES):
        b, hh = divmod(c, 2)
        sl = slice(hh * T, (hh + 1) * T)
        m = dict(shared)
        m["xqT"] = np.ascontiguousarray(qe[b, sl, :].T)
        m["xkT"] = np.ascontiguousarray(ke[b, sl, :].T)
        m["xvT"] = np.ascontiguousarray(ve[b, sl, :].T)
        in_maps.append(m)
    return in_maps


def _run(inputs, trace=False):
    from concourse.bass_utils import run_bass_kernel_spmd

    if "nc" not in _NC_CACHE:
        _NC_CACHE["nc"] = _build_nc()
    nc = _NC_CACHE["nc"]
    in_maps = _prep_inputs(inputs)
    res = run_bass_kernel_spmd(nc, in_maps, core_ids=list(range(NCORES)),
                               trace=trace)
    out = np.empty((B, NSEQ, E), np.float32)
    for c in range(NCORES):
        b, hh = divmod(c, 2)
        out[b, hh * T:(hh + 1) * T, :] = res.results[c]["out"]
    out += np.asarray(inputs["bo"], np.float32)  # bo folded on host
    return out, res


def kernel(**inputs):
    out, _ = _run(inputs, trace=False)
    return out


def kernel_traced(**inputs):
    """Like kernel() but also returns (exec_time_ns, trace_path)."""
    import sys, types
    try:
        import antenv
        if "antenv.axon_hooks" not in sys.modules:
            mod = types.ModuleType("antenv.axon_hooks")
            _h = [None]
            mod.set_axon_ntff_profile_hook = lambda h: _h.__setitem__(0, h)
            mod.get_axon_ntff_profile_hook = lambda: _h[0]
            sys.modules["antenv.axon_hooks"] = mod
            antenv.axon_hooks = mod
            from trn_agent_boot.trn_boot import _ntff_profile_via_ctypes
            mod.set_axon_ntff_profile_hook(
                _ntff_profile_via_ctypes("/opt/axon/libaxon_pjrt.so"))
    except Exception as e:  # profiling is best-effort
        print(f"NTFF hook setup failed: {e}")
    out, res = _run(inputs, trace=True)
    tp = res.instructions_and_trace[1] if res.instructions_and_trace else None
    return out, res.exec_time_ns, tp


# revision 35
# speedup vs baseline: 1.2370x; 1.2370x over previous
"""MultiHeadLinearAttention Trainium2 Bass kernel — 8-core SPMD (v2).

Problem (per reference):
  q = elu(LN(Xq @ Wq.T + bq)) + 1 ; k = elu(LN(Xk @ Wk.T + bk)) + 1
  v = Xv @ Wv.T + bv
  kv = sum_n k[n] (x) v[n]   (per head, [D,D]);  ksum = sum_n k[n]
  out = ((q @ kv) / (q . ksum + 1e-8)) @ Wo.T + bo

Sharding: core c -> batch b = c//2, token half h = c%2 (2048 q AND k/v
tokens each). Per-pair (cores 2b, 2b+1) AllReduce of kv/ksum partials
(~266 KB) completes the sum over all 4096 k/v tokens of the batch.

Layouts on chip (per core):
  k,v: [tok x feat] (LN over free dim; kv contraction over token partitions)
  q:   [feat x tok] (q^T feeds num = kv_bd^T @ q^T and out-proj lhsT)
LayerNorm mean is folded into the weights on host (W~ = W^T(I-J/E),
b~ = b - mean(b)); gq/gk==1, betaq/betak==0 (asserted) so
LN(y) = (y - mu(y)) * rsqrt(var + eps) = u * exp(-0.5*ln(mean(u^2)+eps)).
elu(z)+1 = exp(min(z,0)) + relu(z).

v2 structure (vs v1): Phase A uses [128,512] PSUM proj chunks (bufs=2)
and per-tile rstd so elu/kv pipeline per 128-token tile; the q
projection (B1, AR-independent) is emitted entirely before any
AR-dependent op so the AllReduce hides under it; B2 (elu+den+num) and
the out-projection C are pipelined per 512-token chunk with the den
division fused into the num PSUM evacuation.

All matmuls run as float32r (FP32 bits read at FP22 precision, full PE
rate at moving-dim >= 256).
"""

import os

import numpy as np

B, NSEQ, E, H, D = 4, 4096, 1024, 16, 64
NCORES = 8
T = NSEQ // 2          # tokens per core
TT = T // 128          # token tiles (16)
EI = E // 128          # feature tiles (8)
S = 4                  # B-phase token chunks
SC = T // S            # 512 tokens per chunk
LN_EPS = 1e-5

_NC_CACHE = {}


def _build_nc():
    from concourse import bacc
    import concourse.bass as bass
    import concourse.mybir as mybir
    import concourse.tile as tile

    f32 = mybir.dt.float32
    f32r = mybir.dt.float32r
    Alu = mybir.AluOpType
    Act = mybir.ActivationFunctionType
    RG = [[0, 1], [2, 3], [4, 5], [6, 7]]

    nc = bacc.Bacc(num_devices=NCORES)

    xqT = nc.dram_tensor("xqT", [E, T], f32r, kind="ExternalInput")
    xkT = nc.dram_tensor("xkT", [E, T], f32r, kind="ExternalInput")
    xvT = nc.dram_tensor("xvT", [E, T], f32r, kind="ExternalInput")
    wqT = nc.dram_tensor("wqT", [E, E], f32r, kind="ExternalInput")
    wkT = nc.dram_tensor("wkT", [E, E], f32r, kind="ExternalInput")
    wvT = nc.dram_tensor("wvT", [E, E], f32r, kind="ExternalInput")
    woT = nc.dram_tensor("woT", [E, E], f32r, kind="ExternalInput")
    bq2d = nc.dram_tensor("bq2d", [128, EI], f32, kind="ExternalInput")
    bkR = nc.dram_tensor("bkR", [1, E], f32r, kind="ExternalInput")
    bvR = nc.dram_tensor("bvR", [1, E], f32r, kind="ExternalInput")
    onesC = nc.dram_tensor("onesC", [128, 1], f32r, kind="ExternalInput")
    e2R = nc.dram_tensor("e2R", [2, 128], f32r, kind="ExternalInput")
    zerosBD = nc.dram_tensor("zerosBD", [128, E], f32r, kind="ExternalInput")
    out_d = nc.dram_tensor("out", [T, E], f32, kind="ExternalOutput")

    xqT_v = xqT.rearrange("(i p) n -> p i n", p=128)
    xkT_v = xkT.rearrange("(i p) n -> p i n", p=128)
    xvT_v = xvT.rearrange("(i p) n -> p i n", p=128)
    wqT_v = wqT.rearrange("(i p) j -> p i j", p=128)
    wkT_v = wkT.rearrange("(i p) j -> p i j", p=128)
    wvT_v = wvT.rearrange("(i p) j -> p i j", p=128)
    woT_v = woT.rearrange("(e p) j -> p e j", p=128)

    with tile.TileContext(nc) as tc:
        with tc.tile_pool(name="const", bufs=1) as cp, \
             tc.tile_pool(name="dram", bufs=1, space="DRAM") as dp:
            onesC_sb = cp.tile([128, 1], f32r, tag="onesC_sb")
            nc.sync.dma_start(out=onesC_sb, in_=onesC[:, :])
            e2_sb = cp.tile([2, 128], f32r, tag="e2_sb")
            nc.sync.dma_start(out=e2_sb, in_=e2R[:, :])
            zrow_sb = cp.tile([1, 520], f32r, tag="zrow_sb")
            nc.sync.dma_start(out=zrow_sb, in_=zerosBD[0:1, 0:520])
            ones_row = cp.tile([1, 128], f32, tag="ones_row")
            nc.vector.memset(ones_row, 1.0)
            onesR = ones_row.bitcast(f32r)
            eps_sb = cp.tile([128, 1], f32, tag="eps_sb")
            nc.vector.memset(eps_sb, LN_EPS)
            eps_row = cp.tile([1, 1], f32, tag="eps_row")
            nc.vector.memset(eps_row, LN_EPS)
            bq_sb = cp.tile([128, EI], f32, tag="bq_sb")
            nc.sync.dma_start(out=bq_sb, in_=bq2d[:, :])
            bk_b = cp.tile([128, E], f32r, tag="bk_b")
            nc.sync.dma_start(out=bk_b, in_=bkR[:, :].to_broadcast([128, E]))
            bv_b = cp.tile([128, E], f32r, tag="bv_b")
            nc.sync.dma_start(out=bv_b, in_=bvR[:, :].to_broadcast([128, E]))
            kvbd = cp.tile([128, E], f32r, tag="kvbd")
            ksum2 = cp.tile([128, 16], f32r, tag="ksum2")
            ar_sb = cp.tile([128, 520], f32, tag="ar_sb")
            cc_in = dp.tile([128, 520], f32, tag="cc_in")
            cc_out = dp.tile([128, 520], f32, tag="cc_out")


            # ============ Phase A: k/v proj + elu + kv, per-tile pipeline =
            with tc.tile_pool(name="paw", bufs=1) as paw, \
                 tc.tile_pool(name="pax", bufs=2) as pax, \
                 tc.tile_pool(name="par", bufs=2) as par, \
                 tc.tile_pool(name="psA", bufs=1, space="PSUM") as psA, \
                 tc.tile_pool(name="psAp", bufs=2, space="PSUM") as psAp:
                wk_sb = paw.tile([128, EI, E], f32r, tag="wk")
                for i in range(EI):
                    nc.sync.dma_start(out=wk_sb[:, i, :], in_=wkT_v[:, i, :])
                wv_sb = paw.tile([128, EI, E], f32r, tag="wv")
                ss_all = paw.tile([128, TT], f32, tag="ss_all")
                rstd_all = paw.tile([128, TT], f32, tag="rstd_all")

                kv_ps = [psA.tile([128, 512], f32, tag=f"kv{q}",
                                  name=f"kv{q}") for q in range(4)]
                ksum_ps = psA.tile([128, 8], f32, tag="ksum")
                for q in range(4):
                    nc.tensor.matmul(kv_ps[q], onesR, zrow_sb[:, 0:512],
                                     start=True, stop=False,
                                     skip_group_check=True)
                nc.tensor.matmul(ksum_ps, onesR, zrow_sb[:, 0:8],
                                 start=True, stop=False, skip_group_check=True)

                XC = 256  # tokens per x chunk (2 tiles)
                for g in range(4):          # rstd batch group: 4 tiles
                    ku_t = {}
                    vu_t = {}
                    for cc in range(2):
                        qq = 2 * g + cc
                        qsl = slice(XC * qq, XC * qq + XC)
                        xk_q = pax.tile([128, EI, XC], f32r, tag="xk")
                        for i in range(EI):
                            nc.sync.dma_start(out=xk_q[:, i, :],
                                              in_=xkT_v[:, i, qsl])
                        xv_q = pax.tile([128, EI, XC], f32r, tag="xv")
                        for i in range(EI):
                            nc.sync.dma_start(out=xv_q[:, i, :],
                                              in_=xvT_v[:, i, qsl])
                        if qq == 0:
                            # wv after first xk/xv so tile-0 k MMs start early
                            for i in range(EI):
                                nc.sync.dma_start(out=wv_sb[:, i, :],
                                                  in_=wvT_v[:, i, :])
                        for tl in range(2):
                            t = 2 * qq + tl
                            tsl = slice(128 * tl, 128 * tl + 128)
                            ku = par.tile([128, E], f32, tag="ku", bufs=5,
                                          name=f"ku{t}")
                            kps = psAp.tile([128, E], f32, tag="kps",
                                            bufs=1)
                            for i in range(EI):
                                for jh in range(2):
                                    js = slice(512 * jh, 512 * jh + 512)
                                    nc.tensor.matmul(kps[:, js],
                                                     xk_q[:, i, tsl],
                                                     wk_sb[:, i, js],
                                                     start=(i == 0),
                                                     stop=(i == EI - 1))
                            nc.vector.scalar_tensor_tensor(
                                out=ku, in0=kps, scalar=1.0,
                                in1=bk_b.bitcast(f32),
                                op0=Alu.mult, op1=Alu.add)
                            vu = par.tile([128, E], f32r, tag="vu", bufs=5,
                                          name=f"vu{t}")
                            for jh in range(2):
                                js = slice(512 * jh, 512 * jh + 512)
                                vps = psAp.tile([128, 512], f32, tag="vps",
                                                bufs=1)
                                for i in range(EI):
                                    nc.tensor.matmul(vps, xv_q[:, i, tsl],
                                                     wv_sb[:, i, js],
                                                     start=(i == 0),
                                                     stop=(i == EI - 1))
                                nc.vector.scalar_tensor_tensor(
                                    out=vu[:, js], in0=vps,
                                    scalar=1.0, in1=bv_b.bitcast(f32)[:, js],
                                    op0=Alu.mult, op1=Alu.add)
                            scrap = par.tile([128, E], f32, tag="scrap",
                                             bufs=1)
                            nc.scalar.activation(out=scrap, in_=ku,
                                                 func=Act.Square,
                                                 accum_out=ss_all[:, t:t + 1])
                            ku_t[t] = ku
                            vu_t[t] = vu
                    gs = slice(4 * g, 4 * g + 4)
                    nc.scalar.activation(out=rstd_all[:, gs],
                                         in_=ss_all[:, gs], func=Act.Ln,
                                         scale=1.0 / E, bias=eps_sb)
                    nc.scalar.activation(out=rstd_all[:, gs],
                                         in_=rstd_all[:, gs], func=Act.Exp,
                                         scale=-0.5)
                    for t in range(4 * g, 4 * g + 4):
                        ku = ku_t[t]
                        vu = vu_t[t]
                        rs = rstd_all[:, t:t + 1]
                        km = par.tile([128, E], f32, tag="km")
                        nc.vector.tensor_scalar(out=km, in0=ku, scalar1=rs,
                                                scalar2=0.0, op0=Alu.mult,
                                                op1=Alu.min)
                        nc.scalar.activation(out=km, in_=km, func=Act.Exp)
                        kf = par.tile([128, E], f32r, tag="kf", bufs=3)
                        nc.scalar.activation(out=kf, in_=ku, func=Act.Relu,
                                             scale=rs)
                        nc.vector.tensor_tensor(out=kf, in0=kf.bitcast(f32),
                                                in1=km, op=Alu.add)
                        for q4 in range(4):
                            vq = vu[:, 256 * q4:256 * q4 + 256]
                            for hf in range(2):
                                pr = 2 * q4 + hf
                                kp = kf[:, 128 * pr:128 * pr + 128]
                                nc.tensor.matmul(
                                    kv_ps[q4][:, 256 * hf:256 * hf + 256],
                                    kp, vq, start=False, stop=(t == TT - 1),
                                    skip_group_check=True)
                                nc.tensor.matmul(
                                    ksum_ps[:, pr:pr + 1], kp.bitcast(f32),
                                    onesC_sb.bitcast(f32),
                                    start=False, stop=(t == TT - 1),
                                    skip_group_check=True)

                pack = paw.tile([128, 520], f32, tag="pack")
                for p in range(8):
                    q4, odd = divmod(p, 2)
                    c = 64 * p
                    if odd == 0:
                        nc.vector.tensor_copy(out=pack[0:64, c:c + 64],
                                              in_=kv_ps[q4][0:64, 0:64])
                        nc.vector.tensor_copy(out=pack[64:128, c:c + 64],
                                              in_=kv_ps[q4][64:128, 64:128])
                    else:
                        nc.vector.tensor_copy(out=pack[0:64, c:c + 64],
                                              in_=kv_ps[q4][0:64, 384:448])
                        nc.vector.tensor_copy(out=pack[64:128, c:c + 64],
                                              in_=kv_ps[q4][64:128, 448:512])
                nc.vector.tensor_copy(out=pack[:, 512:520], in_=ksum_ps[:, :])
                nc.sync.dma_start(out=cc_in, in_=pack)

            nc.gpsimd.collective_compute(
                "AllReduce", Alu.add, replica_groups=RG,
                ins=[cc_in[:, :]], outs=[cc_out[:, :]])

            # ============ Phase B1: q proj + LN stats (AR-independent) ====
            # u tiles per (j, s) so B2 writes and C reads stay chunk-local.
            with tc.tile_pool(name="pu", bufs=1) as pu:
                u_t = [[pu.tile([128, SC], f32r, tag=f"u{j}_{s}",
                                name=f"u{j}_{s}") for s in range(S)]
                       for j in range(EI)]
                rb_t = [pu.tile([128, SC], f32, tag=f"rb{s}", name=f"rb{s}")
                        for s in range(S)]
                wo_sb = pu.tile([128, EI, E], f32r, tag="wo")

                with tc.tile_pool(name="pbw", bufs=1) as pbw, \
                     tc.tile_pool(name="pbx", bufs=2) as pbx, \
                     tc.tile_pool(name="pb1", bufs=2) as pb1, \
                     tc.tile_pool(name="psB1", bufs=2, space="PSUM") as psB1:
                    wq_sb = pbw.tile([128, EI, E], f32r, tag="wq")
                    for i in range(EI):
                        nc.sync.dma_start(out=wq_sb[:, i, :],
                                          in_=wqT_v[:, i, :])
                    ssq_list = []
                    for s in range(S):
                        ssl = slice(SC * s, SC * s + SC)
                        xq_s = pbx.tile([128, EI, SC], f32r, tag="xq")
                        for i in range(EI):
                            nc.sync.dma_start(out=xq_s[:, i, :],
                                              in_=xqT_v[:, i, ssl])
                        if s == 1:
                            # wo loads during B1 so C(0) doesn't wait on it
                            for e in range(EI):
                                nc.sync.dma_start(out=wo_sb[:, e, :],
                                                  in_=woT_v[:, e, :])
                        ssq_ps = psB1.tile([1, 512], f32, tag=f"ssq{s}",
                                           bufs=1, name=f"ssq{s}")
                        for j in range(EI):
                            qps = psB1.tile([128, 512], f32, tag="qps")
                            for i in range(EI):
                                nc.tensor.matmul(
                                    qps, wq_sb[:, i, 128 * j:128 * j + 128],
                                    xq_s[:, i, :], start=(i == 0),
                                    stop=(i == EI - 1))
                            u = u_t[j][s]
                            nc.vector.tensor_scalar_add(
                                out=u, in0=qps,
                                scalar1=bq_sb[:, j:j + 1])
                            usq = pb1.tile([128, 512], f32r, tag="usq")
                            nc.scalar.activation(out=usq,
                                                 in_=qps, func=Act.Square,
                                                 bias=bq_sb[:, j:j + 1])
                            nc.tensor.matmul(ssq_ps, onesC_sb, usq,
                                             start=(j == 0),
                                             stop=(j == EI - 1),
                                             skip_group_check=True)
                        ssq_list.append(ssq_ps)
                    # batched rstd for all chunks: one Ln/Exp table trip,
                    # then PE K=1 broadcast to 128 partitions
                    ln_row = pb1.tile([1, T], f32, tag="ln_row", bufs=1)
                    rr_all = pb1.tile([1, T], f32r, tag="rr_all", bufs=1)
                    for s in range(S):
                        nc.scalar.activation(out=ln_row[:, SC * s:SC * s + SC],
                                             in_=ssq_list[s], func=Act.Ln,
                                             scale=1.0 / E, bias=eps_row)
                    nc.scalar.activation(out=rr_all, in_=ln_row,
                                         func=Act.Exp, scale=-0.5)
                    for s in range(S):
                        ssl = slice(SC * s, SC * s + SC)
                        rb_ps = psB1.tile([128, 512], f32, tag="rbps",
                                          bufs=2)
                        nc.tensor.matmul(rb_ps, onesR, rr_all[:, ssl],
                                         start=True, stop=True)
                        nc.vector.tensor_copy(out=rb_t[s], in_=rb_ps)

                # ====== AR unpack (first AR-dependent ops) ======
                nc.gpsimd.dma_start(out=ar_sb, in_=cc_out[:, :])
                nc.gpsimd.dma_start(out=kvbd, in_=zerosBD[:, :])
                nc.gpsimd.dma_start(out=ksum2, in_=zerosBD[:, 0:16])
                ev_dst = kvbd[0:64, :].rearrange("p (a two c) -> p a two c",
                                                 two=2, c=64)[:, :, 0, :]
                nc.vector.tensor_copy(
                    out=ev_dst,
                    in_=ar_sb[0:64, 0:512].rearrange(
                        "p (a c) -> p a c", c=64).bitcast(f32r))
                od_dst = kvbd[64:128, :].rearrange("p (a two c) -> p a two c",
                                                   two=2, c=64)[:, :, 1, :]
                nc.vector.tensor_copy(
                    out=od_dst,
                    in_=ar_sb[64:128, 0:512].rearrange(
                        "p (a c) -> p a c", c=64).bitcast(f32r))
                for jj in range(EI):
                    nc.vector.tensor_copy(
                        out=ksum2[0:64, 2 * jj:2 * jj + 1],
                        in_=ar_sb[0:64, 512 + jj:513 + jj].bitcast(f32r))
                    nc.vector.tensor_copy(
                        out=ksum2[64:128, 2 * jj + 1:2 * jj + 2],
                        in_=ar_sb[64:128, 512 + jj:513 + jj].bitcast(f32r))

                # ============ Phase B2 + C: per-chunk pipeline ============
                with tc.tile_pool(name="pc2", bufs=3) as pc2, \
                     tc.tile_pool(name="psB2", bufs=2, space="PSUM") as psB2:
                    for s in range(S):
                        for j in range(EI):
                            u = u_t[j][s]
                            qf = u.bitcast(f32)
                            nc.vector.tensor_tensor(out=u, in0=qf,
                                                    in1=rb_t[s], op=Alu.mult)
                            m = pc2.tile([128, SC], f32, tag="m")
                            nc.vector.tensor_scalar_min(out=m, in0=qf,
                                                        scalar1=0.0)
                            nc.scalar.activation(out=m, in_=m, func=Act.Exp)
                            nc.scalar.activation(out=u, in_=qf,
                                                 func=Act.Relu)
                            nc.vector.tensor_tensor(out=u, in0=qf, in1=m,
                                                    op=Alu.add)
                            den_ps = psB2.tile([2, 512], f32, tag="dps")
                            nc.tensor.matmul(den_ps, ksum2[:, 2 * j:2 * j + 2],
                                             u, start=True, stop=True)
                            den_r = pc2.tile([2, 512], f32r, tag="denr")
                            nc.vector.tensor_copy(out=den_r, in_=den_ps)
                            drb_ps = psB2.tile([128, 512], f32, tag="drbps")
                            nc.tensor.matmul(drb_ps, e2_sb, den_r,
                                             start=True, stop=True)
                            drb = pc2.tile([128, SC], f32, tag="drb")
                            nc.vector.reciprocal_approx_fast(out=drb,
                                                             in_=drb_ps)
                            num_ps = psB2.tile([128, 512], f32, tag="nps")
                            nc.tensor.matmul(num_ps, kvbd[:, 128 * j:128 * j + 128],
                                             u, start=True, stop=True)
                            nc.vector.tensor_tensor(out=u,
                                                    in0=num_ps, in1=drb,
                                                    op=Alu.mult)
                        for tl in range(4):
                            tsl = slice(128 * tl, 128 * tl + 128)
                            gtsl = slice(SC * s + 128 * tl,
                                         SC * s + 128 * tl + 128)
                            for jh in range(2):
                                js = slice(512 * jh, 512 * jh + 512)
                                ops = psB2.tile([128, 512], f32, tag="ops")
                                for e in range(EI):
                                    nc.tensor.matmul(
                                        ops, u_t[e][s][:, tsl],
                                        wo_sb[:, e, js], start=(e == 0),
                                        stop=(e == EI - 1))
                                osb = pc2.tile([128, 512], f32, tag="osb")
                                nc.scalar.activation(out=osb, in_=ops,
                                                     func=Act.Copy)
                                nc.sync.dma_start(out=out_d[gtsl, js],
                                                  in_=osb)

    nc.finalize()
    return nc


def _prep_inputs(inputs):
    """Host-side fold + per-core shard maps."""
    f = np.float32
    Wq, bq = inputs["Wq"], inputs["bq"]
    Wk, bk = inputs["Wk"], inputs["bk"]
    Wv, bv = inputs["Wv"], inputs["bv"]
    Wo, bo = inputs["Wo"], inputs["bo"]
    for name in ("gq", "gk"):
        assert np.allclose(np.asarray(inputs[name]), 1.0), f"{name} != 1 unsupported"
    for name in ("betaq", "betak"):
        assert np.allclose(np.asarray(inputs[name]), 0.0), f"{name} != 0 unsupported"

    wqT = np.ascontiguousarray(np.asarray(Wq, f).T)
    wqT = wqT - wqT.mean(axis=1, keepdims=True)
    bqf = np.asarray(bq, f) - np.asarray(bq, f).mean()
    wkT = np.ascontiguousarray(np.asarray(Wk, f).T)
    wkT = wkT - wkT.mean(axis=1, keepdims=True)
    bkf = np.asarray(bk, f) - np.asarray(bk, f).mean()
    wvT = np.ascontiguousarray(np.asarray(Wv, f).T)
    woT = np.ascontiguousarray(np.asarray(Wo, f).T)

    shared = {
        "wqT": np.ascontiguousarray(wqT, f),
        "wkT": np.ascontiguousarray(wkT, f),
        "wvT": wvT,
        "woT": woT,
        "bq2d": np.ascontiguousarray(bqf.reshape(EI, 128).T, f),
        "bkR": np.ascontiguousarray(bkf.reshape(1, E), f),
        "bvR": np.ascontiguousarray(np.asarray(bv, f).reshape(1, E)),
        "onesC": np.ones((128, 1), f),
        "e2R": np.ascontiguousarray(
            np.repeat(np.eye(2, dtype=f), 64, axis=1)),
        "zerosBD": np.zeros((128, E), f),
    }
    qe = np.asarray(inputs["query_embed"], f)
    ke = np.asarray(inputs["key_embed"], f)
    ve = np.asarray(inputs["value"], f)
    in_maps = []
    for c in range(NCORES):
        b, hh = divmod(c, 2)
        sl = slice(hh * T, (hh + 1) * T)
        m = dict(shared)
        m["xqT"] = np.ascontiguousarray(qe[b, sl, :].T)
        m["xkT"] = np.ascontiguousarray(ke[b, sl, :].T)
        m["xvT"] = np.ascontiguousarray(ve[b, sl, :].T)
        in_maps.append(m)
    return in_maps


def _run(inputs, trace=False):
    from concourse.bass_utils import run_bass_kernel_spmd

    if "nc" not in _NC_CACHE:
        _NC_CACHE["nc"] = _build_nc()
    nc = _NC_CACHE["nc"]
    in_maps = _prep_inputs(inputs)
    res = run_bass_kernel_spmd(nc, in_maps, core_ids=list(range(NCORES)),
                               trace=trace)
    out = np.empty((B, NSEQ, E), np.float32)
    for c in range(NCORES):
        b, hh = divmod(c, 2)
        out[b, hh * T:(hh + 1) * T, :] = res.results[c]["out"]
    out += np.asarray(inputs["bo"], np.float32)  # bo folded on host
    return out, res


def kernel(**inputs):
    out, _ = _run(inputs, trace=False)
    return out


def kernel_traced(**inputs):
    """Like kernel() but also returns (exec_time_ns, trace_path)."""
    import sys, types
    try:
        import antenv
        if "antenv.axon_hooks" not in sys.modules:
            mod = types.ModuleType("antenv.axon_hooks")
            _h = [None]
            mod.set_axon_ntff_profile_hook = lambda h: _h.__setitem__(0, h)
            mod.get_axon_ntff_profile_hook = lambda: _h[0]
            sys.modules["antenv.axon_hooks"] = mod
            antenv.axon_hooks = mod
            from trn_agent_boot.trn_boot import _ntff_profile_via_ctypes
            mod.set_axon_ntff_profile_hook(
                _ntff_profile_via_ctypes("/opt/axon/libaxon_pjrt.so"))
    except Exception as e:  # profiling is best-effort
        print(f"NTFF hook setup failed: {e}")
    out, res = _run(inputs, trace=True)
    tp = res.instructions_and_trace[1] if res.instructions_and_trace else None
    return out, res.exec_time_ns, tp
